# revision 1
# baseline (speedup 1.0000x reference)
"""Trainium2 Bass kernel for nn_CrossAttention (FFT-query cross attention).

Math:
  out = softmax((Re(FFT(query, axis=1)) @ Wq^T + bq) @ (key @ Wk^T + bk)^T / sqrt(D)) @ key

Key identities used:
  * Re(FFT(x))[j] = sum_n x[n] cos(2*pi*j*n/N): a matmul with a cosine matrix.
  * cos rows satisfy C[N-j] = C[j]  =>  q rows mirror:  q[j] == q[N-j].
    The whole downstream pipeline is row-wise in q, so out[b, j] == out[b, N-j].
    Only rows j = 0..1024 are computed on device (padded to 1152 = 9*128);
    rows 1025..2047 are mirrored from rows 1023..1 on the host.
  * cos cols satisfy C[:, n] = C[:, N-n]  =>  fold x into
    y[0] = x[0], y[n] = x[n] + x[N-n] (n=1..1023), y[1024] = x[1024]
    and contract over only 1025 terms (plus one ones-row for the bq bias).
  * bk drops out of softmax entirely (adds a per-query-row constant to scores).
  * The 1/sqrt(D) scale is folded into the cosine table.
  * 1/rowsum of softmax is applied to the final [128, 256] output tiles, not
    to the [128, 2048] probability tiles.

Per-core layout (core b handles batch b; 8 cores, 8 batches):
  MM-A: z[n, d]   = y @ Wq^T            lhsT = y^T (host),   rhs = Wq^T (host)
  MM-C: kT[d, nk] = Wk @ key^T          lhsT = Wk^T (host),  rhs = key^T (host)
  MM-B: qsT[d, j] = z^T @ (C/16)        lhsT = z,            rhs = cos table (host)
  MM-D: S[j, nk]  = qs @ k^T            lhsT = qsT,          rhs = kT
  softmax rows of S (two 1024-wide halves; exp via ACT with accum_out, P bf16)
  MM-T: P^T tiles via PE transpose (bf16)
  MM-E: o[j, d]   = P @ key             lhsT = P^T (bf16),   rhs = key bf16

Perf notes:
  * Everything scores-side is fp16 (11-bit mantissa, same precision class as
    the PE's f32r mode, but half the DMA bytes and FWL-capable weight loads).
  * P / value side is bf16: bf16 keeps fp32's exponent range, so tiny softmax
    tail probabilities don't flush to zero the way fp16 denormals would.
  * Matmul accumulation chains are interleaved across PSUM banks: consecutive
    PE instructions always target different banks so the drain of one overlaps
    the fill of the next (same-bank accumulation steps serialize).
"""

import numpy as np
import ml_dtypes

import concourse.bass as bass
import concourse.tile as tile
from concourse import bacc, mybir
from concourse.bass_utils import run_bass_kernel_spmd

B = 8
NSEQ = 2048          # query/key sequence length
D = 256              # feature dim
NQH = 1152           # computed query rows (9 tiles of 128; rows >1024 unused)
NFOLD = 1026         # folded contraction: 1025 cosine rows + 1 bias row
NJT = NQH // 128     # 9 query-row tiles
NKT = NSEQ // 128    # 16 key tiles
SCALE = 1.0 / 16.0   # 1/sqrt(D)

f32 = mybir.dt.float32
f32r = mybir.dt.float32r
bf16 = mybir.dt.bfloat16
fp16 = mybir.dt.float16

_compiled = {}


def _build_module():
    nc = bacc.Bacc("TRN2", target_bir_lowering=False, debug=False, num_devices=B)

    dram = {}
    def din(name, shape, dt=f32):
        dram[name] = nc.dram_tensor(name, list(shape), dt, kind="ExternalInput").ap()
    def dout(name, shape):
        dram[name] = nc.dram_tensor(name, list(shape), f32, kind="ExternalOutput").ap()

    din("yt", (D, 1025))          # folded query, transposed
    din("bq", (1, D))
    din("wqt", (D, D))            # Wq^T
    din("wkt", (D, D))            # Wk^T
    din("keyt", (D, NSEQ))        # key^T
    din("keyn", (NSEQ, D), bf16)  # key natural, bf16 (value side)
    din("cth", (NFOLD, NQH), bf16)  # cosine table hi (bf16)
    din("ctl", (NFOLD, NQH), bf16)  # cosine table lo (bf16 residual)
    din("ident", (128, 128), bf16)
    dout("ob", (NQH, D))

    with tile.TileContext(nc) as tc:
        _emit(nc, tc, dram)
    nc.compile()
    return nc


def _emit(nc, tc, dram):
    from contextlib import ExitStack

    with ExitStack() as ctx:
        const = ctx.enter_context(tc.tile_pool(name="const", bufs=1))
        zpool = ctx.enter_context(tc.tile_pool(name="z", bufs=1))
        qkpool = ctx.enter_context(tc.tile_pool(name="qk", bufs=1))

        # ---- constant loads, in phase-consumption order (A, C, B, loop) ----
        yt = [const.tile([128, 1025], f32r, tag=f"yt{i}", name=f"yt{i}") for i in range(2)]
        wqt = [const.tile([128, D], f32r, tag=f"wqt{i}", name=f"wqt{i}") for i in range(2)]
        for i in range(2):
            nc.sync.dma_start(yt[i][:], dram["yt"][i * 128:(i + 1) * 128, :].bitcast(f32r))
            nc.sync.dma_start(wqt[i][:], dram["wqt"][i * 128:(i + 1) * 128, :].bitcast(f32r))
        cts = []
        for i in range(9):
            r = 128 if i < 8 else 2
            th = const.tile([r, NQH], bf16, tag=f"cth{i}", name=f"cth{i}")
            tl = const.tile([r, NQH], bf16, tag=f"ctl{i}", name=f"ctl{i}")
            nc.sync.dma_start(th[:], dram["cth"][i * 128:i * 128 + r, :])
            nc.sync.dma_start(tl[:], dram["ctl"][i * 128:i * 128 + r, :])
            t = const.tile([r, NQH], f32r, tag=f"ct{i}", name=f"ct{i}")
            eng = nc.vector if i % 2 == 0 else nc.gpsimd
            eng.tensor_add(t[:], th[:], tl[:])
            cts.append(t)
        wkt = [const.tile([128, D], f32r, tag=f"wkt{i}", name=f"wkt{i}") for i in range(2)]
        keyt = [const.tile([128, NSEQ], f32r, tag=f"keyt{i}", name=f"keyt{i}") for i in range(2)]
        for i in range(2):
            nc.sync.dma_start(wkt[i][:], dram["wkt"][i * 128:(i + 1) * 128, :].bitcast(f32r))
            nc.sync.dma_start(keyt[i][:], dram["keyt"][i * 128:(i + 1) * 128, :].bitcast(f32r))
        keyn = [const.tile([128, D], bf16, tag=f"keyn{i}", name=f"keyn{i}") for i in range(NKT)]
        for i in range(NKT):
            nc.sync.dma_start(keyn[i][:], dram["keyn"][i * 128:(i + 1) * 128, :])
        id_b = const.tile([128, 128], bf16, tag="ident", name="ident")
        nc.sync.dma_start(id_b[:], dram["ident"][:])

        # ---- phase A: z = y @ Wq^T (9 row tiles; chains interleaved 4-5 wide)
        zbuf = []
        for i in range(8):
            zbuf.append(zpool.tile([128, D], f32r, tag=f"z{i}", name=f"z{i}"))
        zbuf.append(zpool.tile([2, D], f32r, tag="z8", name="z8"))  # row0: z[1024], row1: bq
        nc.sync.dma_start(zbuf[8][1:2, :], dram["bq"][:].bitcast(f32r))

        with tc.tile_pool(name="psA", bufs=5, space="PSUM") as psA:
            for grp in (range(0, 5), range(5, 9)):
                pss = {}
                for nt in grp:
                    pss[nt] = psA.tile([128, D], f32, tag="psA", name="psA")
                for kd in range(2):
                    for nt in grp:
                        m = 128 if nt < 8 else 1
                        nc.tensor.matmul(
                            pss[nt][:m, :], yt[kd][:, nt * 128:nt * 128 + m],
                            wqt[kd][:], start=(kd == 0), stop=(kd == 1))
                for nt in grp:
                    m = 128 if nt < 8 else 1
                    nc.vector.tensor_copy(zbuf[nt][:m, :], pss[nt][:m, :])

        # ---- phases B+C interleaved: qsT = z^T @ (C/16), kT = Wk @ key^T --
        qsT = [qkpool.tile([128, NQH], f32r, tag=f"qsT{i}", name=f"qsT{i}") for i in range(2)]
        kT = [qkpool.tile([128, NSEQ], f32r, tag=f"kT{i}", name=f"kT{i}") for i in range(2)]
        for dt in range(2):
            with tc.tile_pool(name=f"psB{dt}", bufs=3, space="PSUM") as psB, \
                 tc.tile_pool(name=f"psC{dt}", bufs=4, space="PSUM") as psC:
                pb = {c: psB.tile([128, 384], f32, tag="psB", name="psB")
                      for c in range(3)}
                pc = {c: psC.tile([128, 512], f32, tag="psC", name="psC")
                      for c in range(4)}
                # C chains (2 steps) woven between B chain steps (9 steps)
                for kt in range(9):
                    kr = 128 if kt < 8 else 2
                    for c in range(3):
                        sl = slice(c * 384, (c + 1) * 384)
                        nc.tensor.matmul(
                            pb[c][:], zbuf[kt][:kr, dt * 128:(dt + 1) * 128],
                            cts[kt][:kr, sl], start=(kt == 0), stop=(kt == 8))
                    if kt < 2:
                        for c in range(4):
                            sl = slice(c * 512, (c + 1) * 512)
                            nc.tensor.matmul(
                                pc[c][:], wkt[kt][:, dt * 128:(dt + 1) * 128],
                                keyt[kt][:, sl], start=(kt == 0), stop=(kt == 1))
                for c in range(4):
                    sl = slice(c * 512, (c + 1) * 512)
                    nc.vector.tensor_copy(kT[dt][:, sl], pc[c][:])
                for c in range(3):
                    sl = slice(c * 384, (c + 1) * 384)
                    nc.vector.tensor_copy(qsT[dt][:, sl], pb[c][:])

        # ---- phase D: attention over 9 query tiles, software-pipelined ----
        with ExitStack() as jctx:
            psS = jctx.enter_context(tc.tile_pool(name="psS", bufs=2, space="PSUM"))
            psT = jctx.enter_context(tc.tile_pool(name="psT", bufs=2, space="PSUM"))
            psO = jctx.enter_context(tc.tile_pool(name="psO", bufs=2, space="PSUM"))
            work = jctx.enter_context(tc.tile_pool(name="work", bufs=3))
            ptp = jctx.enter_context(tc.tile_pool(name="ptp", bufs=4))
            stats = jctx.enter_context(tc.tile_pool(name="stats", bufs=4))

            state = {}  # per-jt carried tiles
            for step in range(NJT + 2):
                if step >= 2:
                    jt = step - 2
                    p_t, recip = state.pop(jt)
                    # 16 transposes packed 4-per-psum-bank, then one DVE copy
                    # per bank, then the 16 E accumulation steps (2 chains)
                    pt_sbs = []
                    for g in range(4):
                        pt_ps = psT.tile([128, 512], bf16, tag="psT", name="psT",
                                         padded_shape=[128, 1024])
                        for q in range(4):
                            kt = g * 4 + q
                            nc.tensor.matmul(pt_ps[:, q * 128:(q + 1) * 128],
                                             p_t[:, kt * 128:(kt + 1) * 128],
                                             id_b[:], is_transpose=True,
                                             start=True, stop=True)
                        pt_sb = ptp.tile([128, 512], bf16, tag="pt", name="pt")
                        if g % 2 == 0:
                            nc.scalar.copy(pt_sb[:], pt_ps[:])
                        else:
                            nc.vector.tensor_copy(pt_sb[:], pt_ps[:])
                        pt_sbs.append(pt_sb)
                    po = [psO.tile([128, D], f32, tag="psO", name="psO",
                                   padded_shape=[128, 512])
                          for _ in range(2)]
                    for kt in range(NKT):
                        g, q = divmod(kt, 4)
                        nc.tensor.matmul(po[kt % 2][:],
                                         pt_sbs[g][:, q * 128:(q + 1) * 128],
                                         keyn[kt][:],
                                         start=(kt < 2), stop=(kt >= NKT - 2))
                    osb0 = work.tile([128, D], f32, tag="osb0", name="osb0")
                    nc.vector.tensor_scalar_mul(osb0[:], po[0][:], recip[:])
                    osb = work.tile([128, D], f32, tag="osb", name="osb")
                    nc.vector.scalar_tensor_tensor(
                        out=osb[:], in0=po[1][:], scalar=recip[:], in1=osb0[:],
                        op0=mybir.AluOpType.mult, op1=mybir.AluOpType.add)
                    nc.sync.dma_start(dram["ob"][jt * 128:(jt + 1) * 128, :], osb[:])
                if step < NJT:
                    jt = step
                    jsl = slice(jt * 128, (jt + 1) * 128)
                    # scores in two 1024-wide halves (2 psum banks each);
                    # within a half the two 512-chunks interleave the K steps
                    halves = []
                    for h in range(2):
                        sh = psS.tile([128, 1024], f32, tag="psS", name="psS")
                        for dt in range(2):
                            for c in range(2):
                                sl = slice(c * 512, (c + 1) * 512)
                                ksl = slice(h * 1024 + c * 512, h * 1024 + (c + 1) * 512)
                                nc.tensor.matmul(
                                    sh[:, sl], qsT[dt][:, jsl], kT[dt][:, ksl],
                                    start=(dt == 0), stop=(dt == 1))
                        halves.append(sh)
                    mx = [stats.tile([128, 1], f32, tag=f"mx{h}", name=f"mx{h}") for h in range(2)]
                    for h in range(2):
                        nc.vector.reduce_max(out=mx[h][:], in_=halves[h][:],
                                             axis=mybir.AxisListType.X, negate=True)
                    negmax = stats.tile([128, 1], f32, tag="negmax", name="negmax")
                    nc.vector.tensor_scalar_min(negmax[:], mx[0][:], mx[1][:])
                    p_t = work.tile([128, NSEQ], bf16, tag="p", name="p")
                    sm = [stats.tile([128, 1], f32, tag=f"sm{h}", name=f"sm{h}") for h in range(2)]
                    for h in range(2):
                        nc.scalar.activation(
                            out=p_t[:, h * 1024:(h + 1) * 1024], in_=halves[h][:],
                            func=mybir.ActivationFunctionType.Exp,
                            bias=negmax[:], scale=1.0, accum_out=sm[h][:])
                    rsum = stats.tile([128, 1], f32, tag="rsum", name="rsum")
                    nc.vector.tensor_scalar_add(rsum[:], sm[0][:], sm[1][:])
                    recip = stats.tile([128, 1], f32, tag="recip", name="recip")
                    nc.vector.reciprocal(recip[:], rsum[:])
                    state[jt] = (p_t, recip)



def _host_prep(query, key, Wq, bq, Wk, bk):
    """Build per-core input maps (fold+transpose query, transpose key/weights,
    cosine table)."""
    query = np.ascontiguousarray(query, dtype=np.float32)
    key = np.ascontiguousarray(key, dtype=np.float32)

    nn = np.arange(NFOLD - 1, dtype=np.float64)          # 0..1024
    jj = np.arange(NQH, dtype=np.float64)
    ct = np.empty((NFOLD, NQH), dtype=np.float32)
    ct[:-1] = (np.cos(2.0 * np.pi * np.outer(nn, jj) / NSEQ) * SCALE).astype(np.float32)
    ct[-1] = SCALE  # bias row (ones * scale)
    cth = ct.astype(ml_dtypes.bfloat16)
    ctl = (ct - cth.astype(np.float32)).astype(ml_dtypes.bfloat16)

    wqt = np.ascontiguousarray(Wq.T, dtype=np.float32)
    wkt = np.ascontiguousarray(Wk.T, dtype=np.float32)
    bq2 = np.ascontiguousarray(bq.reshape(1, D), dtype=np.float32)

    in_maps = []
    for b in range(B):
        x = query[b]
        y = np.empty((1025, D), dtype=np.float32)
        y[0] = x[0]
        y[1:1024] = x[1:1024] + x[2047:1024:-1]
        y[1024] = x[1024]
        in_maps.append({
            "yt": np.ascontiguousarray(y.T),
            "bq": bq2,
            "wqt": wqt,
            "wkt": wkt,
            "keyt": np.ascontiguousarray(key[b].T),
            "keyn": np.ascontiguousarray(key[b]).astype(ml_dtypes.bfloat16),
            "cth": cth,
            "ctl": ctl,
            "ident": np.eye(128, dtype=ml_dtypes.bfloat16),
        })
    return in_maps


def kernel(query, key, Wq, bq, Wk, bk, _trace=False, _trace_kwargs=None):
    if "nc" not in _compiled:
        _compiled["nc"] = _build_module()
    nc = _compiled["nc"]

    in_maps = _host_prep(query, key, Wq, bq, Wk, bk)
    kw = {}
    if _trace:
        kw["trace"] = True
        if _trace_kwargs:
            kw.update(_trace_kwargs)
    res = run_bass_kernel_spmd(nc, in_maps, core_ids=list(range(B)), **kw)
    _compiled["last_results"] = res

    out = np.empty((B, NSEQ, D), dtype=np.float32)
    for b in range(B):
        ob = res.results[b]["ob"]
        out[b, :1025] = ob[:1025]
        out[b, 1025:] = ob[1023:0:-1]
    return out



# revision 9
# speedup vs baseline: 1.3223x; 1.3223x over previous
"""Trainium2 Bass kernel for nn_CrossAttention (FFT-query cross attention).

Math:
  out = softmax((Re(FFT(query, axis=1)) @ Wq^T + bq) @ (key @ Wk^T + bk)^T / sqrt(D)) @ key

Identities used:
  * Re(FFT(x))[j] = sum_n x[n] cos(2*pi*j*n/N): a matmul with a cosine matrix.
  * Row mirror: out[b, j] == out[b, N-j]; device computes j = 0..1023, host
    computes the single row j=1024 directly and mirrors 1025..2047.
  * Column fold (twice):
      fold1: y[n] = x[n] + x[N-n]           (2048 -> 1025 terms)
      fold2: even j contract yE[n] = y[n]+y[1024-n] vs cos table [513 x 512];
             odd  j contract yO[n] = y[n]-y[1024-n] vs cos table [512 x 512].
    Total cosine table is half of fold1's, and the q-projection matmul halves.
  * bk drops out of softmax (constant per row); bq/16 is added via the qsT
    PSUM drain (per-partition scalar add), so no bias rows in any matmul.
  * 1/sqrt(D) is folded into the cosine table and bias.
  * Wk is folded into the query side: S = (qs @ Wk) @ key^T, so the key
    projection matmul over 2048 rows is replaced by a 256x256 one (qk = qs@Wk).
  * Softmax rowsum comes free out of the P@V matmul via a ones-column
    appended to the value matrix (no ACT accum, no separate reduction).

Per-core layout (core b handles batch b; 8 cores, 8 batches):
  A:  zE/zO[n, d] = y? @ Wq^T           lhsT = yt (host),    rhs = Wq^T (host)
  B:  qsT[d, j]   = z^T @ (C/16) + bq/16  (even|odd j halves; bias in drain)
  C': qkT[d, j]   = Wk^T-contraction of qsT   lhsT = Wk (host), rhs = qsT
  D:  S[j, nk]    = qk @ key^T          lhsT = qkT,          rhs = key^T (host)
      (per j-tile: 4 chunks of 512 keys; chunk max on DVE/Pool as it lands,
       exp per chunk on ACT with global row negmax bias; P in bf16)
  T:  P^T tiles via PE transpose (bf16), interleaved with
  E:  o[j, d]     = P @ [key | 1]       lhsT = P^T chunk,    rhs = key bf16
      col 256 of the accumulators is the softmax row sum; final scale by its
      reciprocal on DVE/Pool.

Scheduling notes:
  * f32r everywhere on the scores side (1 cycle/row when free size >= 256).
  * Per-jt issue order: D(jt) chunks -> softmax(jt) -> T/E(jt-1) interleaved,
    so the PE never waits on the ACT exp chain (chunk-granular pipelining).
  * PSUM: score chunks tag x4 banks, transpose tag x2, output tag x2 = 8.
"""

import numpy as np
import ml_dtypes

import concourse.bass as bass
import concourse.tile as tile
from concourse import bacc, mybir
from concourse.bass_utils import run_bass_kernel_spmd

B = 8
NSEQ = 2048          # query/key sequence length
D = 256              # feature dim
NJ = 1024            # device-computed query rows (512 even + 512 odd classes)
NE = 513             # even-class contraction length
NO = 512             # odd-class contraction length
NKT = NSEQ // 128    # 16 key tiles
SCALE = 1.0 / 16.0   # 1/sqrt(D)

f32 = mybir.dt.float32
f32r = mybir.dt.float32r
bf16 = mybir.dt.bfloat16

_compiled = {}


def _build_module():
    nc = bacc.Bacc("TRN2", target_bir_lowering=False, debug=False, num_devices=B)

    dram = {}
    def din(name, shape, dt=f32):
        dram[name] = nc.dram_tensor(name, list(shape), dt, kind="ExternalInput").ap()
    def dout(name, shape):
        dram[name] = nc.dram_tensor(name, list(shape), f32, kind="ExternalOutput").ap()

    din("yt", (D, 1025))          # [yE^T | yO^T] folded query, transposed
    din("wqt", (D, D))            # Wq^T
    din("bqs", (D, 1))            # bq / 16
    din("cte", (NE, 512))         # cos table, even j classes, * SCALE
    din("cto", (NO, 512))         # cos table, odd j classes, * SCALE
    din("wk", (D, D))             # Wk natural
    din("keyt", (D, NSEQ))        # key^T
    din("keyn", (NSEQ, D + 1), bf16)  # [key | 1] bf16 (value side + rowsum col)
    din("ident", (128, 128), bf16)
    dout("ob", (NJ, D))

    with tile.TileContext(nc) as tc:
        _emit(nc, tc, dram)
    nc.compile()
    return nc


def _emit(nc, tc, dram):
    from contextlib import ExitStack

    X = mybir.AxisListType.X
    EXP = mybir.ActivationFunctionType.Exp

    with ExitStack() as ctx:
        const = ctx.enter_context(tc.tile_pool(name="const", bufs=1))
        zpool = ctx.enter_context(tc.tile_pool(name="z", bufs=1))
        qpool = ctx.enter_context(tc.tile_pool(name="q", bufs=1))

        # ---- constant loads, in phase-consumption order ----
        yt = [const.tile([128, 1025], f32r, tag=f"yt{i}", name=f"yt{i}") for i in range(2)]
        wqt = [const.tile([128, D], f32r, tag=f"wqt{i}", name=f"wqt{i}") for i in range(2)]
        bqs = [const.tile([128, 1], f32, tag=f"bqs{i}", name=f"bqs{i}") for i in range(2)]
        for i in range(2):
            nc.sync.dma_start(yt[i][:], dram["yt"][i * 128:(i + 1) * 128, :].bitcast(f32r))
            nc.sync.dma_start(wqt[i][:], dram["wqt"][i * 128:(i + 1) * 128, :].bitcast(f32r))
            nc.sync.dma_start(bqs[i][:], dram["bqs"][i * 128:(i + 1) * 128, :])
        # cosine tables: interleave E/O so phase B can consume in order
        cte, cto = [], []
        for i in range(5):
            r = 128 if i < 4 else 1
            te = const.tile([r, 512], f32r, tag=f"cte{i}", name=f"cte{i}")
            nc.sync.dma_start(te[:], dram["cte"][i * 128:i * 128 + r, :].bitcast(f32r))
            cte.append(te)
            if i < 4:
                to = const.tile([128, 512], f32r, tag=f"cto{i}", name=f"cto{i}")
                nc.sync.dma_start(to[:], dram["cto"][i * 128:(i + 1) * 128, :].bitcast(f32r))
                cto.append(to)
        wk = [const.tile([128, D], f32r, tag=f"wk{i}", name=f"wk{i}") for i in range(2)]
        for i in range(2):
            nc.sync.dma_start(wk[i][:], dram["wk"][i * 128:(i + 1) * 128, :].bitcast(f32r))
        keyt = [[None] * 4 for _ in range(2)]
        for kc in range(4):
            for dt in range(2):
                t = const.tile([128, 512], f32r, tag=f"keyt{dt}_{kc}", name=f"keyt{dt}_{kc}")
                nc.sync.dma_start(
                    t[:], dram["keyt"][dt * 128:(dt + 1) * 128,
                                       kc * 512:(kc + 1) * 512].bitcast(f32r))
                keyt[dt][kc] = t
        keyn = [const.tile([128, D + 1], bf16, tag=f"keyn{i}", name=f"keyn{i}")
                for i in range(NKT)]
        for i in range(NKT):
            nc.sync.dma_start(keyn[i][:], dram["keyn"][i * 128:(i + 1) * 128, :])
        id_b = const.tile([128, 128], bf16, tag="ident", name="ident")
        nc.sync.dma_start(id_b[:], dram["ident"][:])

        # ---- phase A: zE = yE @ Wq^T (5 tiles), zO = yO @ Wq^T (4 tiles) ----
        # z tiles: 0..4 even-class rows (128,128,128,128,1), 5..8 odd-class.
        zrows = [128, 128, 128, 128, 1, 128, 128, 128, 128]
        zcol0 = [0, 128, 256, 384, 512, 513, 641, 769, 897]
        zbuf = []
        for i in range(9):
            zbuf.append(zpool.tile([zrows[i], D], f32r, tag=f"z{i}", name=f"z{i}"))
        with tc.tile_pool(name="psA", bufs=5, space="PSUM") as psA:
            for grp in (range(0, 5), range(5, 9)):
                pss = {}
                for nt in grp:
                    pss[nt] = psA.tile([128, D], f32, tag="psA", name="psA")
                for kd in range(2):
                    for nt in grp:
                        m = zrows[nt]
                        nc.tensor.matmul(
                            pss[nt][:m, :], yt[kd][:, zcol0[nt]:zcol0[nt] + m],
                            wqt[kd][:], start=(kd == 0), stop=(kd == 1))
                for nt in grp:
                    m = zrows[nt]
                    nc.vector.tensor_copy(zbuf[nt][:m, :], pss[nt][:m, :])

        # ---- phase B: qsT[d, j] = z^T @ C + bq/16 (even | odd halves) ----
        qsT = [qpool.tile([128, NJ], f32r, tag=f"qsT{i}", name=f"qsT{i}") for i in range(2)]
        with tc.tile_pool(name="psB", bufs=1, space="PSUM") as psB:
            pb = {}
            for dt in range(2):
                for h in range(2):  # 0 = even, 1 = odd
                    pb[(dt, h)] = psB.tile([128, 512], f32, tag=f"psB{dt}{h}", name="psB")
            for kt in range(5):
                for dt in range(2):
                    kr = zrows[kt]
                    nc.tensor.matmul(
                        pb[(dt, 0)][:], zbuf[kt][:kr, dt * 128:(dt + 1) * 128],
                        cte[kt][:kr, :], start=(kt == 0), stop=(kt == 4))
                if kt < 4:
                    for dt in range(2):
                        nc.tensor.matmul(
                            pb[(dt, 1)][:], zbuf[5 + kt][:, dt * 128:(dt + 1) * 128],
                            cto[kt][:], start=(kt == 0), stop=(kt == 3))
            for dt in range(2):
                for h in range(2):
                    nc.vector.tensor_scalar_add(
                        qsT[dt][:, h * 512:(h + 1) * 512], pb[(dt, h)][:], bqs[dt][:])

        # ---- phase C': qkT[d, j] = Wk-fold of qsT  (S = qk @ key^T) ----
        qkT = [qpool.tile([128, NJ], f32r, tag=f"qkT{i}", name=f"qkT{i}") for i in range(2)]
        with tc.tile_pool(name="psC", bufs=1, space="PSUM") as psC:
            pc = {}
            for dt in range(2):
                for h in range(2):
                    pc[(dt, h)] = psC.tile([128, 512], f32, tag=f"psC{dt}{h}", name="psC")
            for kd in range(2):
                for dt in range(2):
                    for h in range(2):
                        nc.tensor.matmul(
                            pc[(dt, h)][:], wk[kd][:, dt * 128:(dt + 1) * 128],
                            qsT[kd][:, h * 512:(h + 1) * 512],
                            start=(kd == 0), stop=(kd == 1))
            for dt in range(2):
                for h in range(2):
                    nc.vector.tensor_copy(qkT[dt][:, h * 512:(h + 1) * 512], pc[(dt, h)][:])

        # ---- phase D/T/E: attention over 8 query tiles, chunk-pipelined ----
        with ExitStack() as jctx:
            ps = jctx.enter_context(tc.tile_pool(name="ps", bufs=1, space="PSUM"))
            psO = jctx.enter_context(tc.tile_pool(name="psO", bufs=1, space="PSUM"))
            work = jctx.enter_context(tc.tile_pool(name="work", bufs=1))
            stats = jctx.enter_context(tc.tile_pool(name="stats", bufs=1))

            state = {}
            for step in range(9):
                if step < 8:
                    jt = step
                    jsl = slice(jt * 128, (jt + 1) * 128)
                    # scores: 2 halves of [128, 1024]; half max as each lands
                    scs, ms = [], []
                    for h in range(2):
                        s_h = ps.tile([128, 1024], f32, tag="s", bufs=2, name="s")
                        for c in range(2):
                            kc = h * 2 + c
                            for dt in range(2):
                                nc.tensor.matmul(
                                    s_h[:, c * 512:(c + 1) * 512],
                                    qkT[dt][:, jsl], keyt[dt][kc],
                                    start=(dt == 0), stop=(dt == 1))
                        m = stats.tile([128, 1], f32, tag=f"m{h}", bufs=2, name=f"m{h}")
                        nc.vector.reduce_max(out=m[:], in_=s_h[:], axis=X, negate=True)
                        scs.append(s_h)
                        ms.append(m)
                    nmx = stats.tile([128, 1], f32, tag="nmx", bufs=2, name="nmx")
                    nc.vector.tensor_scalar_min(nmx[:], ms[0][:], ms[1][:])
                    p_t = work.tile([128, NSEQ], bf16, tag="p", bufs=2, name="p")
                    for h in range(2):
                        nc.scalar.activation(
                            out=p_t[:, h * 1024:(h + 1) * 1024], in_=scs[h][:],
                            func=EXP, bias=nmx[:], scale=1.0)
                    state[jt] = p_t
                if step >= 1:
                    jt = step - 1
                    p_t = state.pop(jt)
                    po = psO.tile([128, D + 1], f32, tag="po", bufs=2, name="po",
                                  padded_shape=[128, 512])
                    pt_sbs = {}
                    # PE order: T-half0, T-half1, E 0..7, E 8..15
                    for hg in range(2):
                        pt_ps = ps.tile([128, 1024], bf16, tag="ptps", bufs=2, name="ptps")
                        for q in range(8):
                            kt = hg * 8 + q
                            nc.tensor.matmul(pt_ps[:, q * 128:(q + 1) * 128],
                                             p_t[:, kt * 128:(kt + 1) * 128],
                                             id_b[:], is_transpose=True,
                                             start=True, stop=True)
                        pt_sb = work.tile([128, 1024], bf16, tag="pt", bufs=2, name="pt")
                        if hg == 0:
                            nc.vector.tensor_copy(pt_sb[:], pt_ps[:])
                        else:
                            nc.scalar.copy(pt_sb[:], pt_ps[:])
                        pt_sbs[hg] = pt_sb
                    for kt in range(NKT):
                        nc.tensor.matmul(po[:],
                                         pt_sbs[kt // 8][:, (kt % 8) * 128:(kt % 8 + 1) * 128],
                                         keyn[kt][:],
                                         start=(kt == 0), stop=(kt == NKT - 1))
                    recip = stats.tile([128, 1], f32, tag="recip", bufs=2, name="recip")
                    nc.vector.reciprocal(recip[:], po[:, D:D + 1])
                    osb = work.tile([128, D], f32, tag="osb", bufs=2, name="osb")
                    nc.vector.tensor_scalar_mul(osb[:], po[:, :D], recip[:])
                    nc.sync.dma_start(dram["ob"][jt * 128:(jt + 1) * 128, :], osb[:])


def _host_prep(query, key, Wq, bq, Wk, bk):
    """Per-core input maps: double-folded transposed query, cosine tables,
    transposed key, bf16 [key|1] value matrix."""
    query = np.ascontiguousarray(query, dtype=np.float32)
    key = np.ascontiguousarray(key, dtype=np.float32)

    jE = np.arange(0, NJ, 2, dtype=np.float64)
    jO = np.arange(1, NJ, 2, dtype=np.float64)
    nE = np.arange(NE, dtype=np.float64)
    nO = np.arange(NO, dtype=np.float64)
    cte = (np.cos(2.0 * np.pi * np.outer(nE, jE) / NSEQ) * SCALE).astype(np.float32)
    cto = (np.cos(2.0 * np.pi * np.outer(nO, jO) / NSEQ) * SCALE).astype(np.float32)

    wqt = np.ascontiguousarray(Wq.T, dtype=np.float32)
    wkn = np.ascontiguousarray(Wk, dtype=np.float32)
    bqs = np.ascontiguousarray((np.asarray(bq, dtype=np.float32) * SCALE).reshape(D, 1))
    ident = np.eye(128, dtype=ml_dtypes.bfloat16)

    in_maps = []
    for b in range(B):
        x = query[b]
        y = np.empty((1025, D), dtype=np.float32)
        y[0] = x[0]
        y[1:1024] = x[1:1024] + x[2047:1024:-1]
        y[1024] = x[1024]
        yEO = np.empty((1025, D), dtype=np.float32)
        yEO[0] = y[0] + y[1024]
        yEO[1:512] = y[1:512] + y[1023:512:-1]
        yEO[512] = y[512]
        yEO[513] = y[0] - y[1024]
        yEO[514:1025] = y[1:512] - y[1023:512:-1]
        kb = key[b]
        keyn = np.empty((NSEQ, D + 1), dtype=ml_dtypes.bfloat16)
        keyn[:, :D] = kb
        keyn[:, D] = 1.0
        in_maps.append({
            "yt": np.ascontiguousarray(yEO.T),
            "wqt": wqt,
            "bqs": bqs,
            "cte": cte,
            "cto": cto,
            "wk": wkn,
            "keyt": np.ascontiguousarray(kb.T),
            "keyn": keyn,
            "ident": ident,
        })
    return in_maps


def _host_row1024(query, key, Wq, bq, Wk, bk):
    """Row j=1024 of the output for every batch (one row of attention each),
    plus caches nothing: O(N*D) per batch."""
    rows = np.empty((B, D), dtype=np.float32)
    sgn = ((-1.0) ** np.arange(1025)).astype(np.float64)
    for b in range(B):
        x = query[b].astype(np.float64)
        y = np.empty((1025, D), dtype=np.float64)
        y[0] = x[0]
        y[1:1024] = x[1:1024] + x[2047:1024:-1]
        y[1024] = x[1024]
        q = sgn @ y                                   # Re(FFT) row 1024
        q = q @ np.asarray(Wq, np.float64).T + np.asarray(bq, np.float64)
        kv = key[b].astype(np.float64)
        s = (kv @ (np.asarray(Wk, np.float64).T @ q)
             + np.asarray(bk, np.float64) @ q) * SCALE
        s -= s.max()
        p = np.exp(s)
        rows[b] = (p @ kv / p.sum()).astype(np.float32)
    return rows


def kernel(query, key, Wq, bq, Wk, bk, _trace=False, _trace_kwargs=None):
    if "nc" not in _compiled:
        _compiled["nc"] = _build_module()
    nc = _compiled["nc"]

    in_maps = _host_prep(query, key, Wq, bq, Wk, bk)
    kw = {}
    if _trace:
        kw["trace"] = True
        if _trace_kwargs:
            kw.update(_trace_kwargs)
    res = run_bass_kernel_spmd(nc, in_maps, core_ids=list(range(B)), **kw)
    _compiled["last_results"] = res

    row1024 = _host_row1024(query, key, Wq, bq, Wk, bk)
    out = np.empty((B, NSEQ, D), dtype=np.float32)
    for b in range(B):
        ob = res.results[b]["ob"]
        out[b, 0:NJ:2] = ob[:512]
        out[b, 1:NJ:2] = ob[512:]
        out[b, 1024] = row1024[b]
        out[b, 1025:] = out[b, 1023:0:-1]
    return out
